# revision 1
# baseline (speedup 1.0000x reference)
"""Trainium2 Bass kernel for DifferentiablePointMassSimulator — v2.

Radius recurrence in m-space (m = r^2), serial latency-bound scan:
    G  = A_t * r_t             (DVE, critical path)
    M' = 2*G + T1              (DVE stt, critical path)
    r_{t+1} = Sqrt(M')         (Act)
    T1' = M' + C_{t+1}         (DVE, off critical path)
with A = DT*thrust, C = A^2 + B^2, B = DT*torque.

Everything else (quarter-angle deltas, theta cumsum, sin/cos via
magic-constant rounding, trapezoid positions, output assembly, DMA) is
diced into (time-block x colgroup) chunks and interleaved into the scan's
idle engine time: DVE/Act chunk-ops are "sprinkled" between scan steps,
Pool ops and DMAs drain freely.  The u<0 rationalization of w1 is dropped
(direct w1 keeps rel err ~6e-4 on this dataset vs the 2e-2 gate).

Sharding: pure data parallel, batch 16384 -> 8 cores x 2048; on-core
batch = 128 partitions x 16 columns (b_local = p*16 + col).
"""

import sys

sys.path.insert(0, "/opt/trn_rl_repo")

import numpy as np

import concourse.bass as bass
import concourse.mybir as mybir
from concourse.tile import TileContext

DT = 1.0 / 30.0
P = 128
NB = 16
H = 256
HP = H + 1
S = 8
BC = P * NB
NCORES = 8
B = BC * NCORES

TB = 64
NTB = H // TB
CG = 4
NCG = NB // CG

F32 = mybir.dt.float32
TWO_PI = float(2.0 * np.pi)
MAGIC = float(1.5 * 2 ** 23)
INV_HPI = float(2.0 / np.pi)

_BUILT = None


def build_nc(fixups=True):
    Alu = mybir.AluOpType
    AF = mybir.ActivationFunctionType

    nc = bass.Bass()
    ist = nc.dram_tensor("initial_state", [BC, S], F32, kind="ExternalInput")
    act = nc.dram_tensor("actions", [BC, H, 2], F32, kind="ExternalInput")
    traj = nc.dram_tensor("traj", [BC, H, S], F32, kind="ExternalOutput")

    ist_r = ist.rearrange("(p q) s -> p (q s)", p=P)
    act_r = act.rearrange("(p q) h a -> p (q h a)", p=P)
    traj_r = traj.rearrange("(p q) h s -> p (q h s)", p=P)
    tv3 = traj_r.rearrange("p (b r) -> p b r", b=NB)

    v = nc.vector
    g = nc.gpsimd
    sc = nc.scalar
    sy = nc.sync

    with TileContext(nc) as tc:
        with tc.tile_pool(name="pers", bufs=1) as pp, \
                tc.tile_pool(name="chk", bufs=5) as cp, \
                tc.tile_pool(name="carry", bufs=5) as yp, \
                tc.tile_pool(name="outp", bufs=4) as op:
            RAW = pp.tile([P, NB * H * 2], F32, tag="RAW")
            A = pp.tile([P, NB * H], F32, tag="A")
            Bq = pp.tile([P, NB * H], F32, tag="Bq")
            C = pp.tile([P, NB * H], F32, tag="C")
            RP = pp.tile([P, NB * HP], F32, tag="RP")
            IS = pp.tile([P, NB * S], F32, tag="IS")
            T1 = pp.tile([P, NB * 2], F32, tag="T1")
            GG = pp.tile([P, NB * 2], F32, tag="GG")
            MM = pp.tile([P, NB * 2], F32, tag="MM")
            SMALL = pp.tile([P, NB * 10], F32, tag="SMALL")
            M0 = SMALL[:, 0 * NB:1 * NB]
            W10 = SMALL[:, 1 * NB:2 * NB]
            RMU0 = SMALL[:, 2 * NB:3 * NB]
            MSK0 = SMALL[:, 3 * NB:4 * NB]
            Q0 = SMALL[:, 4 * NB:5 * NB]
            TH0 = SMALL[:, 5 * NB:6 * NB]
            KX = SMALL[:, 6 * NB:7 * NB]
            KY = SMALL[:, 7 * NB:8 * NB]
            GA = SMALL[:, 8 * NB:9 * NB]
            GB = SMALL[:, 9 * NB:10 * NB]

            RAW4 = RAW.rearrange("p (b t a) -> p b t a", b=NB, t=H)
            A3 = A.rearrange("p (b t) -> p b t", b=NB)
            Bq3 = Bq.rearrange("p (b t) -> p b t", b=NB)
            C3 = C.rearrange("p (b t) -> p b t", b=NB)
            RP3 = RP.rearrange("p (b k) -> p b k", b=NB)
            IS3 = IS.rearrange("p (b s) -> p b s", b=NB)
            T13 = T1.rearrange("p (b k) -> p b k", b=NB)
            G3 = GG.rearrange("p (b k) -> p b k", b=NB)
            M3 = MM.rearrange("p (b k) -> p b k", b=NB)

            px0 = IS3[:, :, 0]
            py0 = IS3[:, :, 1]
            vx0 = IS3[:, :, 2]
            vy0 = IS3[:, :, 3]

            # ---------------- loads ----------------
            sy.dma_start(out=IS[:], in_=ist_r[:])
            act4 = act_r.rearrange("p (q h a) -> p q h a", q=NB, h=H)
            for dq in range(4):
                tsl = slice(dq * TB, (dq + 1) * TB)
                sy.dma_start(
                    out=RAW4[:, :, tsl, :],
                    in_=act4[:, :, tsl, :],
                )

            def prep_items(k):
                t0, t1 = k * TB, (k + 1) * TB
                sl = slice(t0, t1)
                thr = RAW4[:, :, sl, 0]
                tor = RAW4[:, :, sl, 1]
                HDT = DT * 0.7071067811865476
                return [
                    ("dve", lambda: v.tensor_scalar(
                        Bq3[:, :, sl], thr, HDT, None, Alu.mult)),
                    ("dve", lambda: v.tensor_mul(
                        Bq3[:, :, sl], Bq3[:, :, sl], Bq3[:, :, sl])),
                    ("dve", lambda: v.tensor_scalar(
                        C3[:, :, sl], tor, HDT, None, Alu.mult)),
                    ("dve", lambda: v.tensor_mul(
                        C3[:, :, sl], C3[:, :, sl], C3[:, :, sl])),
                    ("dve", lambda: v.tensor_add(
                        C3[:, :, sl], C3[:, :, sl], Bq3[:, :, sl])),
                    ("dve", lambda: v.tensor_scalar(
                        A3[:, :, sl], thr, DT, None, Alu.mult)),
                    ("dve", lambda: v.tensor_scalar(
                        Bq3[:, :, sl], tor, DT, None, Alu.mult)),
                ]

            # block 0 prep inline (scan consumes it immediately)
            for _, fn in prep_items(0):
                fn()

            # ---------------- small section ----------------
            sc.activation(GA, vx0, AF.Square)
            sc.activation(GB, vy0, AF.Square)
            v.tensor_add(M0, GA, GB)
            sc.activation(RP3[:, :, 0], M0, AF.Sqrt)
            r0 = RP3[:, :, 0]
            v.tensor_add(W10, r0, vx0)
            v.tensor_sub(RMU0, r0, vx0)
            MSK0i = MSK0.bitcast(mybir.dt.int32)
            v.tensor_scalar(MSK0i, vx0, 0.0, None, Alu.is_lt)
            v.reciprocal(GA, RMU0)
            v.tensor_mul(GA, GB, GA)
            v.copy_predicated(W10, MSK0i, GA)
            v.tensor_mul(GA, r0, W10)
            sc.activation(GA, GA, AF.Sqrt, scale=2.0)
            v.tensor_add(GA, GA, W10)
            v.reciprocal(GA, GA)
            v.tensor_mul(Q0, vy0, GA)
            sc.activation(TH0, Q0, AF.Arctan)
            v.scalar_tensor_tensor(KX, vx0, DT / 2.0, px0, Alu.mult, Alu.add)
            v.scalar_tensor_tensor(KY, vy0, DT / 2.0, py0, Alu.mult, Alu.add)
            v.scalar_tensor_tensor(
                T13[:, :, 0], M0, 0.5, C3[:, :, 0], Alu.mult, Alu.add)

            # ---------------- phase-2 chunks ----------------
            prevTH = [None] * NCG
            prevCX = [None] * NCG
            prevCY = [None] * NCG

            def chunk_items(k, c, dense=False):
                t0, t1 = k * TB, (k + 1) * TB
                cs = slice(c * CG, (c + 1) * CG)
                ch = (slice(None), cs, slice(t0, t1))
                Rt = RP3[:, cs, t0:t1]
                Rp = RP3[:, cs, t0 + 1:t1 + 1]
                NE = CG * TB

                def ct(tag):
                    t = cp.tile([P, NE], F32, tag=tag, name=f"{tag}_{k}_{c}")
                    return t, t.rearrange("p (b t) -> p b t", b=CG)

                def yt(tag):
                    t = yp.tile([P, NE], F32, tag=tag, name=f"{tag}_{k}_{c}")
                    return t, t.rearrange("p (b t) -> p b t", b=CG)

                U, Ux = ct("U")
                W1, W1x = ct("W1")
                W2, W2x = ct("W2")
                DEN, DENx = ct("DEN")
                QC, QCx = ct("QC")
                THc, THx = yt("THc")
                YS, YSx = ct("YS")
                YC, YCx = ct("YC")
                RS, RSx = ct("RS")
                FS, FSx = ct("FS")
                SIN, SINx = ct("SIN")
                COS, COSx = W2, W2x
                VXS, VXSx = W1, W1x
                VYS, VYSx = DEN, DENx
                CX, CXx = yt("CX")
                CY, CYx = yt("CY")
                OUTC = op.tile([P, CG * TB * S], F32, tag="OUTC",
                               name=f"OUTC_{k}_{c}")
                O4 = OUTC.rearrange("p (b t s) -> p b t s", b=CG, t=TB)

                pTH, pCX, pCY = prevTH[c], prevCX[c], prevCY[c]
                prevTH[c], prevCX[c], prevCY[c] = THx, CXx, CYx

                items = []
                ad = lambda fn: items.append(("dve", fn))
                aa = lambda fn: items.append(("act", fn))
                ap_ = lambda fn: items.append(("dve", fn))
                _esel = [0]

                def eng():
                    if not dense:
                        return v
                    _esel[0] ^= 1
                    return v if _esel[0] else g
                ad(lambda: v.tensor_add(Ux[:], Rt, A3[ch]))
                ap_(lambda: eng().tensor_mul(SINx[:], Bq3[ch], Bq3[ch]))
                ad(lambda: v.tensor_add(W1x[:], Ux[:], Rp))
                ad(lambda: v.tensor_sub(W2x[:], Rp, Ux[:]))
                ad(lambda: v.tensor_scalar(
                    YS.bitcast(mybir.dt.int32)[:], U[:], 0.0, None, Alu.is_lt))
                items.append(("stall", [3]))
                ad(lambda: v.reciprocal(W2[:], W2[:]))
                ap_(lambda: eng().tensor_mul(RSx[:], SINx[:], W2x[:]))
                items.append(("stall", [5]))
                ad(lambda: v.copy_predicated(
                    W1[:], YS.bitcast(mybir.dt.int32)[:], RS[:]))
                ap_(lambda: eng().tensor_mul(W2x[:], Rp, W1x[:]))
                items.append(("stall", [8]))
                aa(lambda: sc.activation(W2[:], W2[:], AF.Sqrt, scale=2.0))
                items.append(("stall", [3]))
                ad(lambda: v.tensor_add(DENx[:], W2x[:], W1x[:]))
                ad(lambda: v.reciprocal(DEN[:], DEN[:]))
                ap_(lambda: eng().tensor_mul(QCx[:], Bq3[ch], DENx[:]))
                items.append(("stall", [4]))
                aa(lambda: sc.activation(QC[:], QC[:], AF.Arctan))
                items.append(("split", None))
                items.append(("stall", [3]))
                for j in range(CG):
                    b = c * CG + j
                    init = TH0[:, b:b + 1] if k == 0 else pTH[:, j, TB - 1:TB]
                    ad(lambda j=j, init=init: v.tensor_tensor_scan(
                        THx[:, j, :], QCx[:, j, :], QCx[:, j, :],
                        initial=init, op0=Alu.add, op1=Alu.bypass))
                ad(lambda: v.tensor_scalar(
                    YS[:], THc[:], INV_HPI, None, Alu.mult))
                ad(lambda: v.tensor_scalar(
                    YC[:], THc[:], INV_HPI, 0.25, Alu.mult, Alu.add))
                ad(lambda: v.tensor_scalar(
                    RS[:], YS[:], MAGIC, -MAGIC, Alu.add, Alu.add))
                ap_(lambda: eng().tensor_sub(FS[:], YS[:], RS[:]))
                items.append(("stall", [3]))
                aa(lambda: sc.activation(SIN[:], FS[:], AF.Sin, scale=TWO_PI))
                ad(lambda: v.tensor_scalar(
                    RS[:], YC[:], MAGIC, -MAGIC, Alu.add, Alu.add))
                ap_(lambda: eng().tensor_sub(FS[:], YC[:], RS[:]))
                items.append(("stall", [3]))
                aa(lambda: sc.activation(COS[:], FS[:], AF.Sin, scale=TWO_PI))
                ap_(lambda: eng().tensor_mul(O4[:, :, :, 2], Rp, COSx[:]))
                ap_(lambda: eng().tensor_mul(O4[:, :, :, 3], Rp, SINx[:]))
                items.append(("stall", [3]))
                ad(lambda: v.tensor_scalar(
                    VXSx[:], O4[:, :, :, 2], DT, None, Alu.mult))
                ad(lambda: v.tensor_scalar(
                    VYSx[:], O4[:, :, :, 3], DT, None, Alu.mult))
                for j in range(CG):
                    b = c * CG + j
                    init = KX[:, b:b + 1] if k == 0 else pCX[:, j, TB - 1:TB]
                    ad(lambda j=j, init=init: v.tensor_tensor_scan(
                        CXx[:, j, :], VXSx[:, j, :], VXSx[:, j, :],
                        initial=init, op0=Alu.add, op1=Alu.bypass))
                for j in range(CG):
                    b = c * CG + j
                    init = KY[:, b:b + 1] if k == 0 else pCY[:, j, TB - 1:TB]
                    ad(lambda j=j, init=init: v.tensor_tensor_scan(
                        CYx[:, j, :], VYSx[:, j, :], VYSx[:, j, :],
                        initial=init, op0=Alu.add, op1=Alu.bypass))
                ad(lambda: v.scalar_tensor_tensor(
                    O4[:, :, :, 0], VXSx[:], -0.5, CXx[:], Alu.mult, Alu.add))
                ad(lambda: v.scalar_tensor_tensor(
                    O4[:, :, :, 1], VYSx[:], -0.5, CYx[:], Alu.mult, Alu.add))

                def emit_extras():
                    out_ap = bass.AP(
                        OUTC.tensor, 4,
                        [[CG * TB * S, P], [TB * S, CG], [S, TB], [1, 4]],
                    )
                    in_ap = bass.AP(
                        IS.tensor, c * CG * S + 4,
                        [[NB * S, P], [S, CG], [0, TB], [1, 4]],
                    )
                    (v if dense else g).tensor_copy(out_ap, in_ap)
                ap_(emit_extras)

                def emit_dma():
                    sy.dma_start(
                        out=tv3[:, cs, k * TB * S:(k + 1) * TB * S],
                        in_=OUTC[:],
                    )
                items.append(("dma", emit_dma))
                return items

            # ---------------- scan with sprinkles ----------------
            # Single globally-ordered queue; engine slots pop only a
            # matching HEAD item so program order always respects the
            # producer->consumer emission order.
            q = []

            def enqueue(items):
                q.extend(it for it in items if it[0] != "split")

            for k in range(1, NTB):
                enqueue(prep_items(k))

            pending = [(k, c) for k in range(NTB) for c in range(NCG)]

            def pop(eng):
                # greedily drain any pool/dma run at the head; honour pacing
                # stalls; then at most one item of the requested engine
                while q and q[0][0] in ("pool", "dma"):
                    q.pop(0)[1]()
                if q and q[0][0] == "stall":
                    q[0][1][0] -= 1
                    if q[0][1][0] <= 0:
                        q.pop(0)
                    return
                if q and q[0][0] == eng:
                    q.pop(0)[1]()

            for t in range(H):
                s = t % 2
                ns = 1 - s
                g.tensor_mul(G3[:, :, s], A3[:, :, t], RP3[:, :, t])
                g.tensor_add(M3[:, :, s], G3[:, :, s], T13[:, :, s])
                pop("dve")
                sc.activation(RP3[:, :, t + 1], M3[:, :, s], AF.Sqrt,
                              scale=2.0)
                pop("act")
                if t + 1 < H:
                    g.tensor_add(T13[:, :, ns], M3[:, :, s], C3[:, :, t + 1])
                pop("dve")
                pop("dve")
                pop("act")
                if (t + 1) % 16 == 0 and pending:
                    k, c = pending[0]
                    if k <= (t + 1) // TB - 1:
                        pending.pop(0)
                        enqueue(chunk_items(k, c))

            for k, c in pending:
                enqueue(chunk_items(k, c, dense=True))
            guard = 0
            while q and guard < 100000:
                guard += 1
                pop("dve")
                pop("act")
                pop("dve")
            while q:
                eng, fn = q.pop(0)
                if eng != "stall":
                    fn()

    nc.finalize()
    if fixups:
        _split_multi_waits(nc)
    return nc


def _split_multi_waits(nc):
    """Walrus embeds at most one sync-wait per instruction; split extras onto
    NoOps and drop the tail EVENT_SEMAPHORE_RANGE_CLEAR InstISA."""
    n = 0
    for fn in nc.m.functions:
        for bb in fn.blocks:
            idx = 0
            while idx < len(bb.instructions):
                inst = bb.instructions[idx]
                if (
                    isinstance(inst, mybir.InstISA)
                    and getattr(inst, "op_name", "") == "EVENT_SEMAPHORE_RANGE_CLEAR"
                ):
                    del bb.instructions[idx]
                    continue
                si = getattr(inst, "sync_info", None)
                if si is not None and si.on_wait and len(si.on_wait) >= 2:
                    extra = list(si.on_wait[:-1])
                    keep = list(si.on_wait[-1:])
                    for w in extra:
                        nop = mybir.InstNoOp(
                            name=f"{inst.name}_wsplit{n}", ins=[], outs=[]
                        )
                        n += 1
                        nop.engine = inst.engine
                        nop.sync_info = mybir.SyncInfo(on_wait=[w], on_update=[])
                        bb.instructions.insert(idx, nop)
                        idx += 1
                    inst.sync_info = mybir.SyncInfo(
                        on_wait=keep, on_update=list(si.on_update)
                    )
                idx += 1
    return nc


def _get_built():
    global _BUILT
    if _BUILT is None:
        _BUILT = build_nc()
    return _BUILT


def kernel(initial_state: np.ndarray, actions: np.ndarray) -> np.ndarray:
    from concourse.bass_utils import run_bass_kernel_spmd

    nc = _get_built()
    in_maps = []
    for c in range(NCORES):
        sl = slice(c * BC, (c + 1) * BC)
        in_maps.append(
            {
                "initial_state": np.ascontiguousarray(initial_state[sl]),
                "actions": np.ascontiguousarray(actions[sl]),
            }
        )
    res = run_bass_kernel_spmd(nc, in_maps, core_ids=list(range(NCORES)))
    out = np.concatenate([r["traj"] for r in res.results], axis=0)
    return out



# revision 33
# speedup vs baseline: 1.2118x; 1.2118x over previous
"""Trainium2 Bass kernel for DifferentiablePointMassSimulator — v5.

Radius recurrence in m-space (m = r^2), serial latency-bound scan on the
DVE+Act pair (chain period ~761ns/step vs ~966 for Pool+Act):
    G  = A_t * r_t             (DVE, critical path)
    M' = G + T1                (DVE, critical path)
    r_{t+1} = Sqrt(2*M')       (Act)
    T1' = M' + C_{t+1}         (DVE, runs in the sqrt shadow)
with A = DT*thrust, C = ((DT/sqrt2)*thrust)^2 + ((DT/sqrt2)*torque)^2.

Everything else (quarter-angle deltas, theta cumsum, sin/cos via
magic-constant rounding, trapezoid positions, output assembly, DMA) is
diced into 16 chunks of (32 time steps x 8 columns), all [P,256] pieces:
  - tensor-tensor ops feeding Act activations on DVE, one piece per scan
    step in the ~340ns DVE idle window after T1';
  - activations + scalar ops (sqrt/atan/sin, scale/bias Identity for the
    magic rounding) on Act, max one per step right after the scan sqrt
    (Act idle window ~560ns/step); Act never consumes fresh Pool data
    except THc->YS which is stall-spaced;
  - scans, velocity/position assembly on Pool (off the critical chain,
    ~50% loaded, budget-popped 3/step);
  - input prep (A, B, C=(A^2+B^2)/2, (B/sqrt2)^2) squares/copies on Act,
    tensor-add on Pool;
  - extras channels written once per column group (OUTC SBUF slot pinned
    per c), DMAs on the SP queue drain freely.

Sharding: pure data parallel, batch 16384 -> 8 cores x 2048; on-core
batch = 128 partitions x 16 columns (b_local = p*16 + col).
"""

import sys

sys.path.insert(0, "/opt/trn_rl_repo")

import numpy as np

import concourse.bass as bass
import concourse.mybir as mybir
from concourse.tile import TileContext

DT = 1.0 / 30.0
P = 128
NB = 16
H = 256
HP = H + 1
S = 8
BC = P * NB
NCORES = 8
B = BC * NCORES

TB = 64          # prep dicing block
CG = 8           # chunk column-group width
NCG = NB // CG   # 2
CT = 32          # chunk time width
NCT = H // CT    # 8

F32 = mybir.dt.float32
TWO_PI = float(2.0 * np.pi)
MAGIC = float(1.5 * 2 ** 23)
INV_HPI = float(2.0 / np.pi)

_BUILT = None


def build_nc(fixups=True):
    Alu = mybir.AluOpType
    AF = mybir.ActivationFunctionType

    import os
    debug = os.environ.get("KDBG") == "1"
    scan_only = os.environ.get("KSCAN_ONLY") == "1"
    nc = bass.Bass()
    ist = nc.dram_tensor("initial_state", [BC, S], F32, kind="ExternalInput")
    act = nc.dram_tensor("actions", [BC, H, 2], F32, kind="ExternalInput")
    traj = nc.dram_tensor("traj", [BC, H, S], F32, kind="ExternalOutput")
    dbg = (nc.dram_tensor("dbg", [P, 4 * NB * HP], F32, kind="ExternalOutput")
           if debug else None)

    ist_r = ist.rearrange("(p q) s -> p (q s)", p=P)
    act_r = act.rearrange("(p q) h a -> p (q h a)", p=P)
    traj_r = traj.rearrange("(p q) h s -> p (q h s)", p=P)
    tv3 = traj_r.rearrange("p (b r) -> p b r", b=NB)

    v = nc.vector
    g = nc.gpsimd
    sc = nc.scalar
    sy = nc.sync

    with TileContext(nc) as tc:
        with tc.tile_pool(name="pers", bufs=1) as pp, \
                tc.tile_pool(name="chk", bufs=3) as cp, \
                tc.tile_pool(name="carry", bufs=3) as yp, \
                tc.tile_pool(name="outp", bufs=1) as op:
            RAW = pp.tile([P, NB * H * 2], F32, tag="RAW")
            A = pp.tile([P, NB * H], F32, tag="A")
            Bq = pp.tile([P, NB * H], F32, tag="Bq")
            C = pp.tile([P, NB * H], F32, tag="C")
            BSQH = pp.tile([P, NB * H], F32, tag="BSQH")  # (B/sqrt2)^2
            RP = pp.tile([P, NB * HP], F32, tag="RP")
            RPN = pp.tile([P, NB * HP], F32, tag="RPN")  # -RP
            IS = pp.tile([P, NB * S], F32, tag="IS")
            T1 = pp.tile([P, NB * 2], F32, tag="T1")
            GG = pp.tile([P, NB * 2], F32, tag="GG")
            MM = pp.tile([P, NB * 2], F32, tag="MM")
            SMALL = pp.tile([P, NB * 12], F32, tag="SMALL")
            CONSTS = pp.tile([P, 8], F32, tag="CONSTS")
            g.memset(CONSTS[:, 0:1], 0.25)
            g.memset(CONSTS[:, 1:2], MAGIC)
            g.memset(CONSTS[:, 2:3], -DT / 2.0)
            g.memset(CONSTS[:, 3:4], -2.0)
            g.memset(CONSTS[:, 4:5], -1.0)
            B_QUARTER = CONSTS[:, 0:1]
            B_MAGIC = CONSTS[:, 1:2]

            def cbcast(idx, b, t):
                # [P, b, t] 0-stride broadcast view of CONSTS[:, idx]
                return bass.AP(CONSTS.tensor, idx, [[8, P], [0, b], [0, t]])
            M0 = SMALL[:, 0 * NB:1 * NB]
            W10 = SMALL[:, 1 * NB:2 * NB]
            RMU0 = SMALL[:, 2 * NB:3 * NB]
            MSK0 = SMALL[:, 3 * NB:4 * NB]
            Q0 = SMALL[:, 4 * NB:5 * NB]
            TH0 = SMALL[:, 5 * NB:6 * NB]
            KX = SMALL[:, 6 * NB:7 * NB]
            KY = SMALL[:, 7 * NB:8 * NB]
            GA = SMALL[:, 8 * NB:9 * NB]
            GB = SMALL[:, 9 * NB:10 * NB]
            KXn = SMALL[:, 10 * NB:11 * NB]
            KYn = SMALL[:, 11 * NB:12 * NB]

            RAW4 = RAW.rearrange("p (b t a) -> p b t a", b=NB, t=H)
            A3 = A.rearrange("p (b t) -> p b t", b=NB)
            Bq3 = Bq.rearrange("p (b t) -> p b t", b=NB)
            C3 = C.rearrange("p (b t) -> p b t", b=NB)
            BS3 = BSQH.rearrange("p (b t) -> p b t", b=NB)
            RP3 = RP.rearrange("p (b k) -> p b k", b=NB)
            RPN3 = RPN.rearrange("p (b k) -> p b k", b=NB)
            IS3 = IS.rearrange("p (b s) -> p b s", b=NB)
            T13 = T1.rearrange("p (b k) -> p b k", b=NB)
            G3 = GG.rearrange("p (b k) -> p b k", b=NB)
            M3 = MM.rearrange("p (b k) -> p b k", b=NB)

            px0 = IS3[:, :, 0]
            py0 = IS3[:, :, 1]
            vx0 = IS3[:, :, 2]
            vy0 = IS3[:, :, 3]

            # ---------------- loads ----------------
            sy.dma_start(out=IS[:], in_=ist_r[:])
            act4 = act_r.rearrange("p (q h a) -> p q h a", q=NB, h=H)
            for ta, tb in [(0, 8), (8, 24), (24, 64), (64, 128), (128, 192),
                           (192, 256)]:
                tsl = slice(ta, tb)
                sy.dma_start(
                    out=RAW4[:, :, tsl, :],
                    in_=act4[:, :, tsl, :],
                )

            HDT = DT * 0.7071067811865476

            def prep_items(t0, t1, inline=False):
                """Prep for time range [t0, t1) diced into 4 group quarters:
                BSQH=(tor*HDT)^2, C=(thr*HDT)^2+BSQH, A=thr*DT, Bq=tor*DT.
                Squares/copies on Act, add on Pool; the inline (head) range
                runs on DVE+Pool immediately."""
                sl = slice(t0, t1)
                items = []
                for q in range(4):
                    gs = slice(4 * q, 4 * q + 4)
                    thr = RAW4[:, gs, sl, 0]
                    tor = RAW4[:, gs, sl, 1]
                    ch = (slice(None), gs, sl)
                    if inline:
                        e = v if q % 2 == 0 else g
                        items.append((None, lambda ch=ch, tor=tor:
                                      sc.activation(BS3[ch], tor, AF.Square,
                                                    scale=HDT)))
                        items.append((None, lambda ch=ch, thr=thr:
                                      sc.activation(C3[ch], thr, AF.Square,
                                                    scale=HDT)))
                        items.append((None, lambda ch=ch: v.tensor_add(
                            C3[ch], C3[ch], BS3[ch])))
                        items.append((None, lambda ch=ch, thr=thr:
                                      sc.activation(A3[ch], thr, AF.Copy,
                                                    scale=DT)))
                        items.append((None, lambda ch=ch, tor=tor:
                                      sc.activation(Bq3[ch], tor, AF.Copy,
                                                    scale=DT)))
                    elif os.environ.get("KPREP_DVE") == "1":
                        items.append(("dve", lambda ch=ch, tor=tor:
                                      v.tensor_scalar(BS3[ch], tor, HDT, None,
                                                      Alu.mult)))
                        items.append(("dve", lambda ch=ch: v.tensor_mul(
                            BS3[ch], BS3[ch], BS3[ch])))
                        items.append(("dve", lambda ch=ch, thr=thr:
                                      v.tensor_scalar(C3[ch], thr, HDT, None,
                                                      Alu.mult)))
                        items.append(("dve", lambda ch=ch: v.tensor_mul(
                            C3[ch], C3[ch], C3[ch])))
                        items.append(("dve", lambda ch=ch: v.tensor_add(
                            C3[ch], C3[ch], BS3[ch])))
                        items.append(("dve", lambda ch=ch, thr=thr:
                                      v.tensor_scalar(A3[ch], thr, DT, None,
                                                      Alu.mult)))
                        items.append(("dve", lambda ch=ch, tor=tor:
                                      v.tensor_scalar(Bq3[ch], tor, DT, None,
                                                      Alu.mult)))
                    else:
                        items.append(("act", lambda ch=ch, tor=tor:
                                      sc.activation(BS3[ch], tor, AF.Square,
                                                    scale=HDT)))
                        items.append(("act", lambda ch=ch, thr=thr:
                                      sc.activation(C3[ch], thr, AF.Square,
                                                    scale=HDT)))
                        items.append(("dve", lambda ch=ch: v.tensor_add(
                            C3[ch], C3[ch], BS3[ch])))
                        items.append(("act", lambda ch=ch, thr=thr:
                                      sc.activation(A3[ch], thr, AF.Copy,
                                                    scale=DT)))
                        items.append(("act", lambda ch=ch, tor=tor:
                                      sc.activation(Bq3[ch], tor, AF.Copy,
                                                    scale=DT)))
                return items

            # head prep inline (scan consumes it immediately), smallest
            # time range first so the scan can start early
            for _, fn in prep_items(0, 8, inline=True):
                fn()

            # ---------------- small section ----------------
            sc.activation(GA, vx0, AF.Square)
            sc.activation(GB, vy0, AF.Square)
            v.tensor_add(M0, GA, GB)
            sc.activation(RP3[:, :, 0], M0, AF.Sqrt)
            r0 = RP3[:, :, 0]
            v.tensor_add(W10, r0, vx0)
            v.tensor_sub(RMU0, r0, vx0)
            MSK0i = MSK0.bitcast(mybir.dt.int32)
            v.tensor_scalar(MSK0i, vx0, 0.0, None, Alu.is_lt)
            v.reciprocal(GA, RMU0)
            v.tensor_mul(GA, GB, GA)
            v.copy_predicated(W10, MSK0i, GA)
            v.tensor_mul(GA, r0, W10)
            sc.activation(GA, GA, AF.Sqrt, scale=2.0)
            v.tensor_add(GA, GA, W10)
            v.reciprocal(GA, GA)
            v.tensor_mul(Q0, vy0, GA)
            sc.activation(TH0, Q0, AF.Arctan)
            v.scalar_tensor_tensor(KX, vx0, DT / 2.0, px0, Alu.mult, Alu.add)
            v.scalar_tensor_tensor(KY, vy0, DT / 2.0, py0, Alu.mult, Alu.add)
            v.tensor_scalar(KXn, KX, -0.5, None, Alu.mult)
            v.tensor_scalar(KYn, KY, -0.5, None, Alu.mult)
            v.scalar_tensor_tensor(
                T13[:, :, 0], M0, 0.5, C3[:, :, 0], Alu.mult, Alu.add)

            # ---------------- phase-2 chunks ----------------
            prevTH = [None] * NCG
            prevCX = [None] * NCG
            prevCY = [None] * NCG
            OUTCS = [op.tile([P, CG * CT * S], F32, tag=f"OUTC{c}",
                             name=f"OUTC{c}")
                     for c in range(NCG)]

            def chunk_items(t0, t1, c):
                """Angle/position/output pipeline for time range [t0, t1)
                and column group c (CG=8 columns)."""
                L = t1 - t0
                kc0 = t0 == 0
                cs = slice(c * CG, (c + 1) * CG)
                ch = (slice(None), cs, slice(t0, t1))
                Rt = RP3[:, cs, t0:t1]
                Rp = RP3[:, cs, t0 + 1:t1 + 1]
                NE = CG * L

                def ct(tag):
                    t = cp.tile([P, NE], F32, tag=f"{tag}{L}",
                                name=f"{tag}_{t0}_{c}")
                    return t, t.rearrange("p (b t) -> p b t", b=CG)

                def yt(tag):
                    t = yp.tile([P, NE], F32, tag=f"{tag}{L}",
                                name=f"{tag}_{t0}_{c}")
                    return t, t.rearrange("p (b t) -> p b t", b=CG)

                U, Ux = ct("U")
                W1, W1x = ct("W1")
                W2, W2x = ct("W2")
                DEN, DENx = ct("DEN")
                QC, QCx = ct("QC")
                THc, THx = yt("THc")
                YS, YSx = ct("YS")
                YC, YCx = ct("YC")
                RS, RSx = ct("RS")
                FS, FSx = ct("FS")
                SIN, SINx = ct("SIN")
                COS, COSx = W2, W2x
                VXS, VXSx = W1, W1x
                VYS, VYSx = DEN, DENx
                CX, CXx = yt("CX")
                CY, CYx = yt("CY")
                OUTC = OUTCS[c]
                toff = t0 % CT
                O4 = OUTC.rearrange(
                    "p (b t s) -> p b t s", b=CG, t=CT)[:, :, toff:toff + L, :]

                pTH, pCX, pCY = prevTH[c], prevCX[c], prevCY[c]
                prevTH[c], prevCX[c], prevCY[c] = THx, CXx, CYx

                items = []
                ad = lambda fn: items.append(("dve", fn))
                aa = lambda fn: items.append(("act", fn))
                ap_ = lambda fn: items.append(("pool", fn))

                def adscan3(fns):
                    # bundle tiny scans, two per DVE slot
                    for i in range(0, len(fns), 2):
                        grp = fns[i:i + 2]
                        items.append(("dve", lambda grp=grp: [f() for f in grp]))

                ad(lambda: v.tensor_add(Ux[:], Rt, A3[ch]))
                ad(lambda: v.tensor_add(W1x[:], Ux[:], Rp))
                ad(lambda: v.tensor_sub(W2x[:], Rp, Ux[:]))
                ad(lambda: v.tensor_scalar(
                    YS.bitcast(mybir.dt.int32)[:], U[:], 0.0, None, Alu.is_lt))
                ad(lambda: v.reciprocal(W2[:], W2[:]))
                ad(lambda: v.scalar_tensor_tensor(
                    RSx[:], BS3[ch], 2.0, W2x[:], Alu.mult, Alu.mult))
                ad(lambda: v.copy_predicated(
                    W1[:], YS.bitcast(mybir.dt.int32)[:], RS[:]))
                ad(lambda: v.tensor_mul(W2x[:], Rp, W1x[:]))
                items.append(("stall", [2]))
                aa(lambda: sc.activation(W2[:], W2[:], AF.Sqrt, scale=2.0))
                items.append(("stall", [2]))
                ad(lambda: v.tensor_add(DENx[:], W2x[:], W1x[:]))
                ad(lambda: v.reciprocal(DEN[:], DEN[:]))
                ad(lambda: v.tensor_mul(QCx[:], Bq3[ch], DENx[:]))
                items.append(("stall", [2]))
                aa(lambda: sc.activation(QC[:], QC[:], AF.Arctan))
                items.append(("stall", [2]))
                scan_fns = []
                for j in range(CG):
                    b = c * CG + j
                    init = TH0[:, b:b + 1] if kc0 else pTH[:, j, -1:]
                    scan_fns.append(lambda j=j, init=init: v.tensor_tensor_scan(
                        THx[:, j, :], QCx[:, j, :], QCx[:, j, :],
                        initial=init, op0=Alu.add, op1=Alu.bypass))
                adscan3(scan_fns)
                items.append(("stall", [2]))
                # YS = theta/4 * 2/pi ; YC = YS + 0.25  (Act, const biases)
                aa(lambda: sc.activation(YS[:], THc[:], AF.Identity,
                                         scale=INV_HPI))
                aa(lambda: sc.activation(YC[:], YS[:], AF.Identity,
                                         bias=B_QUARTER))
                # magic rounding: Act adds MAGIC; DVE stt folds (-MAGIC, -YS)
                # producing -FS; the sign is absorbed via the negated radius
                # tile RPN in the Rp*sin muls below.
                aa(lambda: sc.activation(RS[:], YS[:], AF.Identity,
                                         bias=B_MAGIC))
                items.append(("stall", [1]))
                ad(lambda: v.scalar_tensor_tensor(
                    FS[:], RS[:], -MAGIC, YS[:], Alu.add, Alu.subtract))
                items.append(("stall", [1]))
                aa(lambda: sc.activation(SIN[:], FS[:], AF.Sin, scale=TWO_PI))
                aa(lambda: sc.activation(RS[:], YC[:], AF.Identity,
                                         bias=B_MAGIC))
                items.append(("stall", [1]))
                ad(lambda: v.scalar_tensor_tensor(
                    FS[:], RS[:], -MAGIC, YC[:], Alu.add, Alu.subtract))
                items.append(("stall", [1]))
                aa(lambda: sc.activation(COS[:], FS[:], AF.Sin, scale=TWO_PI))
                items.append(("stall", [2]))
                # vel channels: O4[...,2] = Rp*cos, O4[...,3] = Rp*sin
                # (SIN/COS hold negated values; -1 scalar fixes the sign)
                ad(lambda: v.scalar_tensor_tensor(
                    O4[:, :, :, 2], COSx[:], -1.0, Rp, Alu.mult, Alu.mult))
                ad(lambda: v.scalar_tensor_tensor(
                    O4[:, :, :, 3], SINx[:], -1.0, Rp, Alu.mult, Alu.mult))
                ad(lambda: v.tensor_scalar(
                    VXSx[:], O4[:, :, :, 2], DT, None, Alu.mult))
                ad(lambda: v.tensor_scalar(
                    VYSx[:], O4[:, :, :, 3], DT, None, Alu.mult))
                scan_fns = []
                for j in range(CG):
                    b = c * CG + j
                    init = KX[:, b:b + 1] if kc0 else pCX[:, j, -1:]
                    scan_fns.append(lambda j=j, init=init: v.tensor_tensor_scan(
                        CXx[:, j, :], VXSx[:, j, :], VXSx[:, j, :],
                        initial=init, op0=Alu.add, op1=Alu.bypass))
                for j in range(CG):
                    b = c * CG + j
                    init = KY[:, b:b + 1] if kc0 else pCY[:, j, -1:]
                    scan_fns.append(lambda j=j, init=init: v.tensor_tensor_scan(
                        CYx[:, j, :], VYSx[:, j, :], VYSx[:, j, :],
                        initial=init, op0=Alu.add, op1=Alu.bypass))
                adscan3(scan_fns)
                ad(lambda: v.scalar_tensor_tensor(
                    O4[:, :, :, 0], VXSx[:], -0.5, CXx[:], Alu.mult, Alu.add))
                ad(lambda: v.scalar_tensor_tensor(
                    O4[:, :, :, 1], VYSx[:], -0.5, CYx[:], Alu.mult, Alu.add))

                if kc0:
                    def emit_extras():
                        out_ap = bass.AP(
                            OUTC.tensor, 4,
                            [[CG * CT * S, P], [CT * S, CG], [S, CT], [1, 4]],
                        )
                        in_ap = bass.AP(
                            IS.tensor, c * CG * S + 4,
                            [[NB * S, P], [S, CG], [0, CT], [1, 4]],
                        )
                        g.tensor_copy(out_ap, in_ap)
                    items.append(("pool", emit_extras))
                    assert L == CT

                OUTCv = OUTC.rearrange("p (b ts) -> p b ts", b=CG)

                def emit_dma():
                    sy.dma_start(
                        out=tv3[:, cs, t0 * S:t1 * S],
                        in_=OUTCv[:, :, toff * S:(toff + L) * S],
                    )
                items.append(("dma", emit_dma))
                return items

            # chunk schedule: (ready_step, t0, t1, c)
            sched = []
            for sk in range(NCT - 1):
                for c in range(NCG):
                    sched.append((CT * (sk + 1), CT * sk, CT * (sk + 1), c))
            for t0s, t1s in [(224, 240), (240, 256)]:
                for c in range(NCG):
                    sched.append((t1s, t0s, t1s, c))

            # ---------------- scan with interleaved work ----------------
            q = []

            def enqueue(items):
                q.extend(items)

            prepq = []  # (deadline_step, tag, fn)
            for t0p, t1p in [(8, 24), (24, 64)] + [
                    (k * TB, (k + 1) * TB) for k in range(1, H // TB)]:
                for tag, fn in prep_items(t0p, t1p):
                    prepq.append((t0p, tag, fn))

            pending = list(sched)

            def pop(eng, tick=False):
                while q and q[0][0] == "dma":
                    q.pop(0)[1]()
                if q and q[0][0] == "stall":
                    if tick:
                        q[0][1][0] -= 1
                        if q[0][1][0] <= 0:
                            q.pop(0)
                    return
                if q and q[0][0] == eng:
                    q.pop(0)[1]()
                    while q and q[0][0] == "dma":
                        q.pop(0)[1]()

            if scan_only:
                pending = []
            last_trigger = [-100]
            for t in range(H):
                s = t % 2
                ns = 1 - s
                while prepq and prepq[0][0] <= t + 3:
                    prepq.pop(0)[2]()
                if prepq and (t % 2 == 0):
                    prepq.pop(0)[2]()
                g.tensor_mul(G3[:, :, s], A3[:, :, t], RP3[:, :, t])
                g.tensor_add(M3[:, :, s], G3[:, :, s], T13[:, :, s])
                sc.activation(RP3[:, :, t + 1], M3[:, :, s], AF.Sqrt,
                              scale=2.0)
                pop("act")
                pop("dve", tick=True)
                pop("dve")
                if t + 1 < H:
                    g.tensor_add(T13[:, :, ns], M3[:, :, s],
                                 C3[:, :, t + 1])
                pop("pool")
                pop("pool")
                pop("pool")
                spacing = 14 if pending and pending[0][2] - pending[0][1] == CT else 7
                if (pending and pending[0][0] <= t + 1
                        and t + 1 - last_trigger[0] >= spacing):
                    last_trigger[0] = t + 1
                    rdy, t0c, t1c, c = pending.pop(0)
                    enqueue(chunk_items(t0c, t1c, c))

            if debug:
                sy.dma_start(out=dbg[:, 0:NB * HP], in_=RP[:])
                sy.dma_start(out=dbg[:, NB * HP:2 * NB * HP - NB],
                             in_=A[:, :NB * (HP - 1)])
                sy.dma_start(out=dbg[:, 2 * NB * HP:2 * NB * HP + NB * H],
                             in_=C[:])
                sy.dma_start(out=dbg[:, 3 * NB * HP:3 * NB * HP + NB * H],
                             in_=Bq[:])
            for _, _, fn in prepq:
                fn()
            prepq.clear()
            # post-scan: no chain to protect — emit everything, the
            # remaining chunks round-robin interleaved so engines overlap
            tail_lists = [chunk_items(t0c, t1c, c)
                          for rdy, t0c, t1c, c in pending]
            ti = [0] * len(tail_lists)
            while any(ti[i] < len(tl) for i, tl in enumerate(tail_lists)):
                for i, tl in enumerate(tail_lists):
                    if ti[i] < len(tl):
                        eng, fn = tl[ti[i]]
                        ti[i] += 1
                        if eng != "stall":
                            q.append((eng, fn))
            for eng, fn in q:
                if eng != "stall":
                    fn()
            q.clear()

    nc.finalize()
    if fixups:
        _split_multi_waits(nc)
    return nc


def _split_multi_waits(nc):
    """Walrus embeds at most one sync-wait per instruction; split extras onto
    NoOps and drop the tail EVENT_SEMAPHORE_RANGE_CLEAR InstISA."""
    n = 0
    for fn in nc.m.functions:
        for bb in fn.blocks:
            idx = 0
            while idx < len(bb.instructions):
                inst = bb.instructions[idx]
                if (
                    isinstance(inst, mybir.InstISA)
                    and getattr(inst, "op_name", "") == "EVENT_SEMAPHORE_RANGE_CLEAR"
                ):
                    del bb.instructions[idx]
                    continue
                si = getattr(inst, "sync_info", None)
                if si is not None and si.on_wait and len(si.on_wait) >= 2:
                    extra = list(si.on_wait[:-1])
                    keep = list(si.on_wait[-1:])
                    for w in extra:
                        nop = mybir.InstNoOp(
                            name=f"{inst.name}_wsplit{n}", ins=[], outs=[]
                        )
                        n += 1
                        nop.engine = inst.engine
                        nop.sync_info = mybir.SyncInfo(on_wait=[w], on_update=[])
                        bb.instructions.insert(idx, nop)
                        idx += 1
                    inst.sync_info = mybir.SyncInfo(
                        on_wait=keep, on_update=list(si.on_update)
                    )
                idx += 1
    return nc


def _get_built():
    global _BUILT
    if _BUILT is None:
        _BUILT = build_nc()
    return _BUILT


def kernel(initial_state: np.ndarray, actions: np.ndarray) -> np.ndarray:
    from concourse.bass_utils import run_bass_kernel_spmd

    nc = _get_built()
    in_maps = []
    for c in range(NCORES):
        sl = slice(c * BC, (c + 1) * BC)
        in_maps.append(
            {
                "initial_state": np.ascontiguousarray(initial_state[sl]),
                "actions": np.ascontiguousarray(actions[sl]),
            }
        )
    res = run_bass_kernel_spmd(nc, in_maps, core_ids=list(range(NCORES)))
    out = np.concatenate([r["traj"] for r in res.results], axis=0)
    return out
